# revision 15
# baseline (speedup 1.0000x reference)
"""DiffPool-style GNN message passing on 8 trn2 NeuronCores.

Reference computation (N=4096, F_IN=512, F_OUT=4096):
    h = relu(x @ W1 + b1)            [N, F_OUT]
    s = relu(x @ W2 + b2)            [N, F_OUT]
    a = exp(h @ W3 + b3) * adj       [N, N]
    a = a / rowsum(a)
    out = a @ s                      [N, F_OUT]

Sharding: 1D row-parallel over 8 cores (512 rows each). Each core computes
its row-shard of h, s, a. The full s (needed as the RHS of a @ s) is
assembled with an on-chip AllGather of the per-core s shards, overlapped
with the big h @ W3 GEMM.

v2 design notes (all driven by the cost-model timeline sim):
- All four GEMMs run in fp8e4 DoubleRow (2x PE throughput). Weights are
  host-prescaled into fp8's normal range; the scales are undone inside the
  fused activations. Row normalization is applied by scaling output rows
  with 1/rowsum (mathematically identical).
- Weights/adj are host-preblocked so every load DMA is one contiguous run
  per partition (128 descriptors instead of 512-1024) and whole column
  blocks arrive in a single DMA: 56 DMAs per iteration instead of 158.
- DMA issue is spread over the three DGE-capable queues (SP + Activation
  HWDGE for loads, Pool SWDGE for stores) so one queue's head-of-line
  wait cannot stall unrelated prefetches.
- PSUM is organized as one pool of [128, 4, 512] fp32 tiles (4 banks per
  slot, 2 slots = all 8 banks). Each matmul group writes one 512-wide
  bank slice; activations then process all 4 slices in a single batched
  instruction (zero-bias fast path).
"""

import numpy as np
import ml_dtypes

import concourse.bass as bass
import concourse.mybir as mybir
import concourse.tile as tile
from concourse import bacc
from concourse.bass import ts
from concourse.bass_utils import run_bass_kernel_spmd

BF = mybir.dt.bfloat16
F32 = mybir.dt.float32
F8 = mybir.dt.float8e4
W3_SCALE = 64.0  # W3 values (~1e-3 std) are rescaled into fp8e4's normal range
W1_SCALE = 16.0  # W1 likewise; hT then holds 16*h, undone in the exp scale
W2_SCALE = 16.0  # W2 likewise; psum holds 16*(x@W2), undone in the relu scale

N = 4096
F_IN = 512
F_OUT = 4096
NCORES = 8
R = N // NCORES  # 512 rows per core

KT_IN = F_IN // 128    # 4 k-tiles for the x-side GEMMs
KT_BIG = F_OUT // 128  # 32 k-tiles for the big GEMMs
NB = F_OUT // 512      # 8 column blocks of 512
RT = R // 128          # 4 row tiles per shard

AFT = mybir.ActivationFunctionType
ALU = mybir.AluOpType


def build_nc(loop_reps: int | None = None, with_b2: bool = True,
             parts: str = "bacd", fp8_d: bool = True,
             fp8_b: bool = True, body_reps: int | None = None,
             zero_bias: bool = True) -> bass.Bass:
    """loop_reps: timing-only variant — wraps the compute stages in a dynamic
    For_i loop (collective hoisted out, since collectives can't sit inside
    control flow) so per-iteration time can be measured past the ~100ms
    axon dispatch jitter. body_reps: sim-only unrolled bodies, no collective.
    """
    if with_b2:
        fp8_b = False  # bias-matmul path is bf16-only
    nc = bacc.Bacc("TRN2", target_bir_lowering=False, debug=False, num_devices=NCORES)

    # Host-preblocked inputs: every per-partition line is contiguous.
    xT8_d = nc.dram_tensor("xT8", [128, KT_IN, R], F8, kind="ExternalInput")
    w1_d = nc.dram_tensor("w1", [NB, 128, KT_IN, 512], F8, kind="ExternalInput")
    w2_d = nc.dram_tensor("w2", [NB, 128, KT_IN, 512], F8 if fp8_b else BF,
                          kind="ExternalInput")
    w3_d = nc.dram_tensor("w3", [NB, 128, KT_BIG, 512], F8, kind="ExternalInput")
    adjT_d = nc.dram_tensor("adjT", [NB, 128, 4, R], F8, kind="ExternalInput")
    b1_d = nc.dram_tensor("b1", [F_OUT], F32, kind="ExternalInput")
    b2_d = nc.dram_tensor("b2", [F_OUT], F32, kind="ExternalInput")
    b3_d = nc.dram_tensor("b3", [N], F32, kind="ExternalInput")
    # bf16 x (only read when the bf16 stage-B fallback is active)
    xT_d = nc.dram_tensor("xT", [128, KT_IN, R], BF, kind="ExternalInput")
    out_d = nc.dram_tensor("out", [R, F_OUT], BF, kind="ExternalOutput")

    b1_v = b1_d.rearrange("(t p) -> p t", p=128)            # [128, 32]
    b3_v = b3_d.rearrange("(t p) -> p t", p=128)            # [128, 32]
    out_v = out_d.rearrange("(rt p) c -> p rt c", p=128)    # [128, 4, 4096]

    SD = F8 if fp8_d else BF

    with tile.TileContext(nc) as tc:
        with (
            tc.tile_pool(name="const", bufs=1) as constp,
            tc.tile_pool(name="hpool", bufs=1) as hp,
            tc.tile_pool(name="epool", bufs=1) as ep,
            tc.tile_pool(name="w1p", bufs=2) as w1p,
            tc.tile_pool(name="w2p", bufs=2) as w2p,
            tc.tile_pool(name="sB", bufs=2) as sBp,
            tc.tile_pool(name="w3p", bufs=2) as w3p,
            tc.tile_pool(name="adjp", bufs=2) as adjp,
            tc.tile_pool(name="expp", bufs=2) as expp,
            tc.tile_pool(name="sDp", bufs=2) as sDp,
            tc.tile_pool(name="outp", bufs=2) as outp,
            tc.tile_pool(name="psum", bufs=2, space="PSUM") as psump,
            tc.tile_pool(name="dram", bufs=1, space="DRAM") as dramp,
        ):
            # ---- persistent constants / small tensors ----
            xT8_sb = constp.tile([128, KT_IN, R], F8)
            nc.sync.dma_start(xT8_sb[:], xT8_d[:])
            if not fp8_b:
                xT_sb = constp.tile([128, KT_IN, R], BF)
                nc.sync.dma_start(xT_sb[:], xT_d[:])
            b1_sb = constp.tile([128, KT_BIG], F32)
            nc.sync.dma_start(b1_sb[:], b1_v[:])
            b3_sb = constp.tile([128, KT_BIG], F32)
            nc.sync.dma_start(b3_sb[:], b3_v[:])
            if with_b2:
                b2row_f = constp.tile([1, F_OUT], F32)
                nc.sync.dma_start(b2row_f[:], b2_d[None, :])
                b2row = constp.tile([1, F_OUT], BF)
                nc.vector.tensor_copy(b2row[:], b2row_f[:])
                ones_row = constp.tile([1, 128], BF)
                nc.vector.memset(ones_row[:], 1.0)
            ones_col = constp.tile([128, 1], F32)
            nc.vector.memset(ones_col[:], 1.0)
            dAcc = constp.tile([128, R], F32)
            d_row = constp.tile([1, R], F32)
            dT_sb = constp.tile([128, RT], F32)
            rd_sb = constp.tile([128, RT], F32)

            hT_sb = hp.tile([128, KT_BIG, R], F8)   # h transposed  [F_OUT, R]
            eT_sb = ep.tile([128, KT_BIG, R], SD)   # e transposed  [N, R]

            s_in_dram = dramp.tile([R, F_OUT], SD)
            s_out_dram = dramp.tile([N, F_OUT], SD)
            d_dram = dramp.tile([R], F32)
            s_in_v = s_in_dram.rearrange("(rt p) c -> p rt c", p=128)
            s_out_v = s_out_dram.rearrange("(kt p) c -> p kt c", p=128)

            def stage_b_block(cb):
                w2_sb = w2p.tile([128, KT_IN, 512], F8 if fp8_b else BF,
                                 name="w2_sb")
                nc.sync.dma_start(w2_sb[:], w2_d[cb, :, :, :])
                psB4 = psump.tile([128, RT, 512], F32, name="ps", tag="ps")
                s4 = sBp.tile([128, RT, 512], SD, name="s4")
                for rt in range(RT):
                    if fp8_b:
                        for u in range(KT_IN // 2):
                            nc.tensor.matmul(
                                psB4[:, rt, :],
                                xT8_sb[:, 2 * u : 2 * u + 2, ts(rt, 128)],
                                w2_sb[:, 2 * u : 2 * u + 2, :],
                                start=(u == 0),
                                stop=(u == KT_IN // 2 - 1),
                                perf_mode=mybir.MatmulPerfMode.DoubleRow,
                            )
                    else:
                        for kt in range(KT_IN):
                            nc.tensor.matmul(
                                psB4[:, rt, :],
                                xT_sb[:, kt, ts(rt, 128)],
                                w2_sb[:, kt, :],
                                start=(kt == 0),
                                stop=(not with_b2 and kt == KT_IN - 1),
                            )
                        if with_b2:
                            nc.tensor.matmul(
                                psB4[:, rt, :],
                                ones_row[:],
                                b2row[:, ts(cb, 512)],
                                start=False,
                                stop=True,
                            )
                # one batched relu over all four bank slices
                nc.scalar.activation(
                    s4[:], psB4[:], AFT.Relu,
                    scale=(1.0 / W2_SCALE) if fp8_b else 1.0,
                )
                nc.gpsimd.dma_start(s_in_v[:, :, ts(cb, 512)], s4[:])

            def stage_a_block(fg):
                # hT = relu(x_i @ (16*W1))^T = 16*h^T; fp8 DoubleRow
                w1_sb = w1p.tile([128, KT_IN, 512], F8, name="w1_sb")
                nc.scalar.dma_start(w1_sb[:], w1_d[fg, :, :, :])
                psA4 = psump.tile([128, 4, 512], F32, name="ps", tag="ps")
                for fw in range(4):
                    for u in range(KT_IN // 2):
                        nc.tensor.matmul(
                            psA4[:, fw, :],
                            w1_sb[:, 2 * u : 2 * u + 2, ts(fw, 128)],
                            xT8_sb[:, 2 * u : 2 * u + 2, :],
                            start=(u == 0),
                            stop=(u == KT_IN // 2 - 1),
                            perf_mode=mybir.MatmulPerfMode.DoubleRow,
                        )
                if zero_bias:
                    # relu on DVE (Act is the BA-phase bottleneck; DVE is idle)
                    nc.vector.tensor_scalar_max(hT_sb[:, ts(fg, 4), :], psA4[:], 0.0)
                else:
                    for fw in range(4):
                        ft = fg * 4 + fw
                        nc.scalar.activation(
                            hT_sb[:, ft, :], psA4[:, fw, :], AFT.Relu,
                            bias=b1_sb[:, ft : ft + 1],
                        )

            def stage_b():
                for cb in range(NB):
                    stage_b_block(cb)

            def stage_a():
                for fg in range(NB):
                    stage_a_block(fg)

            def stage_ba():
                for blk in range(NB):
                    stage_b_block(blk)
                    stage_a_block(blk)

            def all_gather():
                nc.gpsimd.collective_compute(
                    "AllGather",
                    ALU.bypass,
                    replica_groups=[list(range(NCORES))],
                    ins=[s_in_dram[:]],
                    outs=[s_out_dram[:]],
                )

            def stage_c():
                # eT = (exp(h @ W3 + b3) * adj)^T  [N, R]; dAcc accumulation
                for cb in range(NB):
                    w3_sb = w3p.tile([128, KT_BIG, 512], F8, name="w3_sb")
                    nc.sync.dma_start(w3_sb[:], w3_d[cb, :, :, :])
                    adj_sb = adjp.tile([128, 4, R], F8, name="adj_sb")
                    nc.sync.dma_start(adj_sb[:], adjT_d[cb, :, :, :])
                    psC4 = psump.tile([128, 4, 512], F32, name="ps", tag="ps")
                    for cw in range(4):
                        NP = KT_BIG // 2
                        for u in range(NP):
                            nc.tensor.matmul(
                                psC4[:, cw, :],
                                w3_sb[:, 2 * u : 2 * u + 2, ts(cw, 128)],
                                hT_sb[:, 2 * u : 2 * u + 2, :],
                                start=(u == 0),
                                stop=(u == NP - 1),
                                perf_mode=mybir.MatmulPerfMode.DoubleRow,
                            )
                    ex4 = expp.tile([128, 4, R], BF, name="ex4")
                    if zero_bias:
                        # one batched exp over all four bank slices (b3 == 0)
                        nc.scalar.activation(
                            ex4[:], psC4[:], AFT.Exp,
                            scale=1.0 / (W3_SCALE * W1_SCALE),
                        )
                    else:
                        for cw in range(4):
                            ct = cb * 4 + cw
                            nc.scalar.activation(
                                ex4[:, cw, :], psC4[:, cw, :], AFT.Exp,
                                bias=b3_sb[:, ct : ct + 1],
                                scale=1.0 / (W3_SCALE * W1_SCALE),
                            )
                    nc.vector.tensor_tensor(
                        eT_sb[:, ts(cb, 4), :], ex4[:], adj_sb[:], op=ALU.mult
                    )
                    for cw in range(4):
                        ct = cb * 4 + cw
                        if ct == 0:
                            nc.vector.tensor_copy(dAcc[:], eT_sb[:, 0, :])
                        else:
                            nc.vector.tensor_tensor(
                                dAcc[:], dAcc[:], eT_sb[:, ct, :], op=ALU.add
                            )

                # rowsums: reduce dAcc over partitions with a ones matmul,
                # round-trip through DRAM to relayout [1,512] -> [128,4]
                psD4 = psump.tile([128, 4, 512], F32, name="ps", tag="ps")
                psD1 = psD4[0:1, 0, :]  # [1, R] view; keeps pool slots uniform
                nc.tensor.matmul(psD1, ones_col[:], dAcc[:], start=True, stop=True)
                nc.scalar.copy(d_row[:], psD1)
                nc.sync.dma_start(d_dram.rearrange("(a r) -> a r", a=1), d_row[:])
                nc.sync.dma_start(dT_sb[:], d_dram.rearrange("(t p) -> p t", p=128))
                nc.vector.reciprocal(rd_sb[:], dT_sb[:])

            def stage_d():
                # out_i = diag(1/d) (e_i @ s)  [R, F_OUT]
                for cb in range(NB):
                    sD_sb = sDp.tile([128, KT_BIG, 512], SD, name="sD_sb")
                    nc.scalar.dma_start(sD_sb[:], s_out_v[:, :, ts(cb, 512)])
                    psE4 = psump.tile([128, 4, 512], F32, name="ps", tag="ps")
                    ob4 = outp.tile([128, 4, 512], BF, name="ob4")
                    for rt in range(RT):
                        if fp8_d:
                            NP = KT_BIG // 2
                            for u in range(NP):
                                nc.tensor.matmul(
                                    psE4[:, rt, :],
                                    eT_sb[:, 2 * u : 2 * u + 2, ts(rt, 128)],
                                    sD_sb[:, 2 * u : 2 * u + 2, :],
                                    start=(u == 0),
                                    stop=(u == NP - 1),
                                    perf_mode=mybir.MatmulPerfMode.DoubleRow,
                                )
                        else:
                            for kt in range(KT_BIG):
                                nc.tensor.matmul(
                                    psE4[:, rt, :],
                                    eT_sb[:, kt, ts(rt, 128)],
                                    sD_sb[:, kt, :],
                                    start=(kt == 0),
                                    stop=(kt == KT_BIG - 1),
                                )
                        nc.vector.tensor_scalar_mul(
                            ob4[:, rt, :], psE4[:, rt, :], rd_sb[:, rt : rt + 1]
                        )
                    nc.gpsimd.dma_start(out_v[:, :, ts(cb, 512)], ob4[:])

            if body_reps is not None:
                # sim-only: unrolled loop bodies, no collective (timing study)
                for _ in range(body_reps):
                    stage_ba()
                    stage_c()
                    stage_d()
            elif loop_reps is None:
                stage_ba()
                all_gather()
                stage_c()
                stage_d()
            else:
                stage_b()
                all_gather()
                stage_a()
                stage_c()  # so eT/rd are valid even if the loop omits stages
                with tc.For_i(0, loop_reps, 1):
                    if "b" in parts and "a" in parts:
                        stage_ba()
                    elif "b" in parts:
                        stage_b()
                    elif "a" in parts:
                        stage_a()
                    if "c" in parts:
                        stage_c()
                    if "d" in parts:
                        stage_d()

    nc.compile()
    return nc


def make_in_maps(x, adj, W1, b1, W2, b2, W3, b3, fp8_b=True):
    if np.any(np.asarray(b2)):
        fp8_b = False  # mirror build_nc's bias-path fallback
    bf = ml_dtypes.bfloat16
    f8 = ml_dtypes.float8_e4m3
    xT = np.ascontiguousarray(x.T).astype(bf)        # [F_IN, N]
    adjT = np.ascontiguousarray(adj.T)               # [N, N] (cols x rows)
    xT8 = np.clip(xT.astype(np.float32), -240, 240).astype(f8)

    # preblocked weights: [NB][128][kt][512] with contiguous per-partition runs
    def preblock(w, scale, dtype):
        wb = np.clip(np.asarray(w, np.float32) * scale, -240, 240).astype(dtype)
        kt = wb.shape[0] // 128
        return np.ascontiguousarray(
            wb.reshape(kt, 128, NB, 512).transpose(2, 1, 0, 3)
        )

    w1b = preblock(W1, W1_SCALE, f8)
    w2b = preblock(W2, W2_SCALE, f8) if fp8_b else preblock(W2, 1.0, bf)
    w3b = preblock(W3, W3_SCALE, f8)

    b1f = np.ascontiguousarray(b1).astype(np.float32) * np.float32(W1_SCALE)
    b2f = np.ascontiguousarray(b2).astype(np.float32)
    b3f = np.ascontiguousarray(b3).astype(np.float32)

    in_maps = []
    for i in range(NCORES):
        sl = slice(i * R, (i + 1) * R)
        xTc = xT[:, sl]       # [512, 512]
        xT8c = xT8[:, sl]
        adjc = adjT[:, sl].astype(f8)  # [4096, 512]
        in_maps.append(
            {
                # [128, kt, R] preblocked
                "xT": np.ascontiguousarray(
                    xTc.reshape(KT_IN, 128, R).transpose(1, 0, 2)),
                "xT8": np.ascontiguousarray(
                    xT8c.reshape(KT_IN, 128, R).transpose(1, 0, 2)),
                # [NB][128][4][R] preblocked
                "adjT": np.ascontiguousarray(
                    adjc.reshape(NB, 4, 128, R).transpose(0, 2, 1, 3)),
                "w1": w1b,
                "w2": w2b,
                "w3": w3b,
                "b1": b1f,
                "b2": b2f,
                "b3": b3f,
            }
        )
    return in_maps


def run(x, adj, W1, b1, W2, b2, W3, b3, trace=False, fp8_d=True, fp8_b=True):
    zb = not (np.any(np.asarray(b1)) or np.any(np.asarray(b3)))
    nc = build_nc(with_b2=bool(np.any(np.asarray(b2))), fp8_d=fp8_d, fp8_b=fp8_b,
                  zero_bias=zb)
    in_maps = make_in_maps(x, adj, W1, b1, W2, b2, W3, b3, fp8_b=fp8_b)
    res = run_bass_kernel_spmd(nc, in_maps, core_ids=list(range(NCORES)), trace=trace)
    out = np.concatenate([res.results[i]["out"] for i in range(NCORES)], axis=0)
    return out.astype(np.float32), res


def kernel(x, adj, W1, b1, W2, b2, W3, b3):
    args = [np.asarray(a) for a in (x, adj, W1, b1, W2, b2, W3, b3)]
    out, _ = run(*args, trace=False)
    return out


# revision 19
# speedup vs baseline: 3.8163x; 3.8163x over previous
"""DiffPool-style GNN message passing on 8 trn2 NeuronCores.

Reference computation (N=4096, F_IN=512, F_OUT=4096):
    h = relu(x @ W1 + b1)            [N, F_OUT]
    s = relu(x @ W2 + b2)            [N, F_OUT]
    a = exp(h @ W3 + b3) * adj       [N, N]
    a = a / rowsum(a)
    out = a @ s                      [N, F_OUT]

Sharding: 1D row-parallel over 8 cores (512 rows each). Each core computes
its row-shard of h, s, a. The full s (needed as the RHS of a @ s) is
assembled with an on-chip AllGather of the per-core s shards, overlapped
with the big h @ W3 GEMM.

v2 design notes (all driven by the cost-model timeline sim):
- All four GEMMs run in fp8e4 DoubleRow (2x PE throughput). Weights are
  host-prescaled into fp8's normal range; the scales are undone inside the
  fused activations. Row normalization is applied by scaling output rows
  with 1/rowsum (mathematically identical).
- Weights/adj are host-preblocked so every load DMA is one contiguous run
  per partition (128 descriptors instead of 512-1024) and whole column
  blocks arrive in a single DMA: 56 DMAs per iteration instead of 158.
- DMA issue is spread over the three DGE-capable queues (SP + Activation
  HWDGE for loads, Pool SWDGE for stores) so one queue's head-of-line
  wait cannot stall unrelated prefetches.
- PSUM is organized as one pool of [128, 4, 512] fp32 tiles (4 banks per
  slot, 2 slots = all 8 banks). Each matmul group writes one 512-wide
  bank slice; activations then process all 4 slices in a single batched
  instruction (zero-bias fast path).
"""

import numpy as np
import ml_dtypes

import concourse.bass as bass
import concourse.mybir as mybir
import concourse.tile as tile
from concourse import bacc
from concourse.bass import ts
from concourse.bass_utils import run_bass_kernel_spmd

BF = mybir.dt.bfloat16
F32 = mybir.dt.float32
F8 = mybir.dt.float8e4
W3_SCALE = 64.0  # W3 values (~1e-3 std) are rescaled into fp8e4's normal range
W1_SCALE = 16.0  # W1 likewise; hT then holds 16*h, undone in the exp scale
W2_SCALE = 16.0  # W2 likewise; psum holds 16*(x@W2), undone in the relu scale

N = 4096
F_IN = 512
F_OUT = 4096
NCORES = 8
R = N // NCORES  # 512 rows per core

KT_IN = F_IN // 128    # 4 k-tiles for the x-side GEMMs
KT_BIG = F_OUT // 128  # 32 k-tiles for the big GEMMs
NB = F_OUT // 512      # 8 column blocks of 512
RT = R // 128          # 4 row tiles per shard

AFT = mybir.ActivationFunctionType
ALU = mybir.AluOpType


def build_nc(loop_reps: int | None = None, with_b2: bool = True,
             parts: str = "bacd", fp8_d: bool = True,
             fp8_b: bool = True, body_reps: int | None = None,
             zero_bias: bool = True) -> bass.Bass:
    """loop_reps: timing-only variant — wraps the compute stages in a dynamic
    For_i loop (collective hoisted out, since collectives can't sit inside
    control flow) so per-iteration time can be measured past the ~100ms
    axon dispatch jitter. body_reps: sim-only unrolled bodies, no collective.
    """
    if with_b2:
        fp8_b = False  # bias-matmul path is bf16-only
    nc = bacc.Bacc("TRN2", target_bir_lowering=False, debug=False, num_devices=NCORES)

    # Host-preblocked inputs: every per-partition line is contiguous.
    xT8_d = nc.dram_tensor("xT8", [128, KT_IN, R], F8, kind="ExternalInput")
    w1_d = nc.dram_tensor("w1", [NB, 128, KT_IN, 512], F8, kind="ExternalInput")
    w2_d = nc.dram_tensor("w2", [NB, 128, KT_IN, 512], F8 if fp8_b else BF,
                          kind="ExternalInput")
    w3_d = nc.dram_tensor("w3", [NB, 128, KT_BIG, 512], F8, kind="ExternalInput")
    adjT_d = nc.dram_tensor("adjT", [NB, 128, 4, R], F8, kind="ExternalInput")
    b1_d = nc.dram_tensor("b1", [F_OUT], F32, kind="ExternalInput")
    b2_d = nc.dram_tensor("b2", [F_OUT], F32, kind="ExternalInput")
    b3_d = nc.dram_tensor("b3", [N], F32, kind="ExternalInput")
    # bf16 x (only read when the bf16 stage-B fallback is active)
    xT_d = nc.dram_tensor("xT", [128, KT_IN, R], BF, kind="ExternalInput")
    if loop_reps is not None:
        # runtime trip count: one NEFF times any K (timing-only variant)
        reps_d = nc.dram_tensor("reps", [1], mybir.dt.int32, kind="ExternalInput")
    out_d = nc.dram_tensor("out", [R, F_OUT], BF, kind="ExternalOutput")

    b1_v = b1_d.rearrange("(t p) -> p t", p=128)            # [128, 32]
    b3_v = b3_d.rearrange("(t p) -> p t", p=128)            # [128, 32]
    out_v = out_d.rearrange("(rt p) c -> p rt c", p=128)    # [128, 4, 4096]

    SD = F8 if fp8_d else BF

    with tile.TileContext(nc) as tc:
        with (
            tc.tile_pool(name="const", bufs=1) as constp,
            tc.tile_pool(name="hpool", bufs=1) as hp,
            tc.tile_pool(name="epool", bufs=1) as ep,
            tc.tile_pool(name="w1p", bufs=2) as w1p,
            tc.tile_pool(name="w2p", bufs=2) as w2p,
            tc.tile_pool(name="sB", bufs=2) as sBp,
            tc.tile_pool(name="w3p", bufs=2) as w3p,
            tc.tile_pool(name="adjp", bufs=2) as adjp,
            tc.tile_pool(name="expp", bufs=2) as expp,
            tc.tile_pool(name="sDp", bufs=2) as sDp,
            tc.tile_pool(name="outp", bufs=2) as outp,
            tc.tile_pool(name="psum", bufs=2, space="PSUM") as psump,
            tc.tile_pool(name="dram", bufs=1, space="DRAM") as dramp,
        ):
            # ---- persistent constants / small tensors ----
            xT8_sb = constp.tile([128, KT_IN, R], F8)
            nc.sync.dma_start(xT8_sb[:], xT8_d[:])
            if not fp8_b:
                xT_sb = constp.tile([128, KT_IN, R], BF)
                nc.sync.dma_start(xT_sb[:], xT_d[:])
            b1_sb = constp.tile([128, KT_BIG], F32)
            nc.sync.dma_start(b1_sb[:], b1_v[:])
            b3_sb = constp.tile([128, KT_BIG], F32)
            nc.sync.dma_start(b3_sb[:], b3_v[:])
            if with_b2:
                b2row_f = constp.tile([1, F_OUT], F32)
                nc.sync.dma_start(b2row_f[:], b2_d[None, :])
                b2row = constp.tile([1, F_OUT], BF)
                nc.vector.tensor_copy(b2row[:], b2row_f[:])
                ones_row = constp.tile([1, 128], BF)
                nc.vector.memset(ones_row[:], 1.0)
            ones_col = constp.tile([128, 1], F32)
            nc.vector.memset(ones_col[:], 1.0)
            dAcc = constp.tile([128, R], F32)
            d_row = constp.tile([1, R], F32)
            dT_sb = constp.tile([128, RT], F32)
            rd_sb = constp.tile([128, RT], F32)

            hT_sb = hp.tile([128, KT_BIG, R], F8)   # h transposed  [F_OUT, R]
            eT_sb = ep.tile([128, KT_BIG, R], SD)   # e transposed  [N, R]

            s_in_dram = dramp.tile([R, F_OUT], SD)
            s_out_dram = dramp.tile([N, F_OUT], SD)
            d_dram = dramp.tile([R], F32)
            s_in_v = s_in_dram.rearrange("(rt p) c -> p rt c", p=128)
            s_out_v = s_out_dram.rearrange("(kt p) c -> p kt c", p=128)

            def stage_b_block(cb):
                w2_sb = w2p.tile([128, KT_IN, 512], F8 if fp8_b else BF,
                                 name="w2_sb")
                nc.sync.dma_start(w2_sb[:], w2_d[cb, :, :, :])
                psB4 = psump.tile([128, RT, 512], F32, name="ps", tag="ps")
                s4 = sBp.tile([128, RT, 512], SD, name="s4")
                for rt in range(RT):
                    if fp8_b:
                        for u in range(KT_IN // 2):
                            nc.tensor.matmul(
                                psB4[:, rt, :],
                                xT8_sb[:, 2 * u : 2 * u + 2, ts(rt, 128)],
                                w2_sb[:, 2 * u : 2 * u + 2, :],
                                start=(u == 0),
                                stop=(u == KT_IN // 2 - 1),
                                perf_mode=mybir.MatmulPerfMode.DoubleRow,
                            )
                    else:
                        for kt in range(KT_IN):
                            nc.tensor.matmul(
                                psB4[:, rt, :],
                                xT_sb[:, kt, ts(rt, 128)],
                                w2_sb[:, kt, :],
                                start=(kt == 0),
                                stop=(not with_b2 and kt == KT_IN - 1),
                            )
                        if with_b2:
                            nc.tensor.matmul(
                                psB4[:, rt, :],
                                ones_row[:],
                                b2row[:, ts(cb, 512)],
                                start=False,
                                stop=True,
                            )
                # one batched relu over all four bank slices
                nc.scalar.activation(
                    s4[:], psB4[:], AFT.Relu,
                    scale=(1.0 / W2_SCALE) if fp8_b else 1.0,
                )
                nc.gpsimd.dma_start(s_in_v[:, :, ts(cb, 512)], s4[:])

            def stage_a_block(fg):
                # hT = relu(x_i @ (16*W1))^T = 16*h^T; fp8 DoubleRow
                w1_sb = w1p.tile([128, KT_IN, 512], F8, name="w1_sb")
                nc.scalar.dma_start(w1_sb[:], w1_d[fg, :, :, :])
                psA4 = psump.tile([128, 4, 512], F32, name="ps", tag="ps")
                for fw in range(4):
                    for u in range(KT_IN // 2):
                        nc.tensor.matmul(
                            psA4[:, fw, :],
                            w1_sb[:, 2 * u : 2 * u + 2, ts(fw, 128)],
                            xT8_sb[:, 2 * u : 2 * u + 2, :],
                            start=(u == 0),
                            stop=(u == KT_IN // 2 - 1),
                            perf_mode=mybir.MatmulPerfMode.DoubleRow,
                        )
                if zero_bias:
                    # relu on DVE (Act is the BA-phase bottleneck; DVE is idle)
                    nc.vector.tensor_scalar_max(hT_sb[:, ts(fg, 4), :], psA4[:], 0.0)
                else:
                    for fw in range(4):
                        ft = fg * 4 + fw
                        nc.scalar.activation(
                            hT_sb[:, ft, :], psA4[:, fw, :], AFT.Relu,
                            bias=b1_sb[:, ft : ft + 1],
                        )

            def stage_b():
                for cb in range(NB):
                    stage_b_block(cb)

            def stage_a():
                for fg in range(NB):
                    stage_a_block(fg)

            def stage_ba():
                for blk in range(NB):
                    stage_b_block(blk)
                    stage_a_block(blk)

            def all_gather():
                nc.gpsimd.collective_compute(
                    "AllGather",
                    ALU.bypass,
                    replica_groups=[list(range(NCORES))],
                    ins=[s_in_dram[:]],
                    outs=[s_out_dram[:]],
                )

            def stage_c():
                # eT = (exp(h @ W3 + b3) * adj)^T  [N, R]; dAcc accumulation
                for cb in range(NB):
                    w3_sb = w3p.tile([128, KT_BIG, 512], F8, name="w3_sb")
                    nc.sync.dma_start(w3_sb[:], w3_d[cb, :, :, :])
                    adj_sb = adjp.tile([128, 4, R], F8, name="adj_sb")
                    nc.sync.dma_start(adj_sb[:], adjT_d[cb, :, :, :])
                    psC4 = psump.tile([128, 4, 512], F32, name="ps", tag="ps")
                    for cw in range(4):
                        NP = KT_BIG // 2
                        for u in range(NP):
                            nc.tensor.matmul(
                                psC4[:, cw, :],
                                w3_sb[:, 2 * u : 2 * u + 2, ts(cw, 128)],
                                hT_sb[:, 2 * u : 2 * u + 2, :],
                                start=(u == 0),
                                stop=(u == NP - 1),
                                perf_mode=mybir.MatmulPerfMode.DoubleRow,
                            )
                    ex4 = expp.tile([128, 4, R], BF, name="ex4")
                    if zero_bias:
                        # one batched exp over all four bank slices (b3 == 0)
                        nc.scalar.activation(
                            ex4[:], psC4[:], AFT.Exp,
                            scale=1.0 / (W3_SCALE * W1_SCALE),
                        )
                    else:
                        for cw in range(4):
                            ct = cb * 4 + cw
                            nc.scalar.activation(
                                ex4[:, cw, :], psC4[:, cw, :], AFT.Exp,
                                bias=b3_sb[:, ct : ct + 1],
                                scale=1.0 / (W3_SCALE * W1_SCALE),
                            )
                    nc.vector.tensor_tensor(
                        eT_sb[:, ts(cb, 4), :], ex4[:], adj_sb[:], op=ALU.mult
                    )
                    for cw in range(4):
                        ct = cb * 4 + cw
                        if ct == 0:
                            nc.vector.tensor_copy(dAcc[:], eT_sb[:, 0, :])
                        else:
                            nc.vector.tensor_tensor(
                                dAcc[:], dAcc[:], eT_sb[:, ct, :], op=ALU.add
                            )

                # rowsums: reduce dAcc over partitions with a ones matmul,
                # round-trip through DRAM to relayout [1,512] -> [128,4]
                psD4 = psump.tile([128, 4, 512], F32, name="ps", tag="ps")
                psD1 = psD4[0:1, 0, :]  # [1, R] view; keeps pool slots uniform
                nc.tensor.matmul(psD1, ones_col[:], dAcc[:], start=True, stop=True)
                nc.scalar.copy(d_row[:], psD1)
                nc.sync.dma_start(d_dram.rearrange("(a r) -> a r", a=1), d_row[:])
                nc.sync.dma_start(dT_sb[:], d_dram.rearrange("(t p) -> p t", p=128))
                nc.vector.reciprocal(rd_sb[:], dT_sb[:])

            def stage_d():
                # out_i = diag(1/d) (e_i @ s)  [R, F_OUT]
                for cb in range(NB):
                    sD_sb = sDp.tile([128, KT_BIG, 512], SD, name="sD_sb")
                    nc.scalar.dma_start(sD_sb[:], s_out_v[:, :, ts(cb, 512)])
                    psE4 = psump.tile([128, 4, 512], F32, name="ps", tag="ps")
                    ob4 = outp.tile([128, 4, 512], BF, name="ob4")
                    for rt in range(RT):
                        if fp8_d:
                            NP = KT_BIG // 2
                            for u in range(NP):
                                nc.tensor.matmul(
                                    psE4[:, rt, :],
                                    eT_sb[:, 2 * u : 2 * u + 2, ts(rt, 128)],
                                    sD_sb[:, 2 * u : 2 * u + 2, :],
                                    start=(u == 0),
                                    stop=(u == NP - 1),
                                    perf_mode=mybir.MatmulPerfMode.DoubleRow,
                                )
                        else:
                            for kt in range(KT_BIG):
                                nc.tensor.matmul(
                                    psE4[:, rt, :],
                                    eT_sb[:, kt, ts(rt, 128)],
                                    sD_sb[:, kt, :],
                                    start=(kt == 0),
                                    stop=(kt == KT_BIG - 1),
                                )
                        nc.vector.tensor_scalar_mul(
                            ob4[:, rt, :], psE4[:, rt, :], rd_sb[:, rt : rt + 1]
                        )
                    nc.gpsimd.dma_start(out_v[:, :, ts(cb, 512)], ob4[:])

            if body_reps is not None:
                # sim-only: unrolled loop bodies, no collective (timing study)
                for _ in range(body_reps):
                    stage_ba()
                    stage_c()
                    stage_d()
            elif loop_reps is None:
                stage_ba()
                all_gather()
                stage_c()
                stage_d()
            else:
                reps_sb = constp.tile([1, 1], mybir.dt.int32)
                nc.sync.dma_start(reps_sb[:], reps_d[None, :])
                _, (reps_val,) = nc.values_load_multi_w_load_instructions(
                    reps_sb[0:1, :], min_val=0, max_val=1 << 20,
                    skip_runtime_bounds_check=True,
                )
                stage_b()
                all_gather()
                stage_a()
                stage_c()  # so eT/rd are valid even if the loop omits stages
                with tc.For_i(0, reps_val, 1):
                    if "b" in parts and "a" in parts:
                        stage_ba()
                    elif "b" in parts:
                        stage_b()
                    elif "a" in parts:
                        stage_a()
                    if "c" in parts:
                        stage_c()
                    if "d" in parts:
                        stage_d()

    nc.compile()
    return nc


def make_in_maps(x, adj, W1, b1, W2, b2, W3, b3, fp8_b=True):
    if np.any(np.asarray(b2)):
        fp8_b = False  # mirror build_nc's bias-path fallback
    bf = ml_dtypes.bfloat16
    f8 = ml_dtypes.float8_e4m3
    xT = np.ascontiguousarray(x.T).astype(bf)        # [F_IN, N]
    adjT = np.ascontiguousarray(adj.T)               # [N, N] (cols x rows)
    xT8 = np.clip(xT.astype(np.float32), -240, 240).astype(f8)

    # preblocked weights: [NB][128][kt][512] with contiguous per-partition runs
    def preblock(w, scale, dtype):
        wb = np.clip(np.asarray(w, np.float32) * scale, -240, 240).astype(dtype)
        kt = wb.shape[0] // 128
        return np.ascontiguousarray(
            wb.reshape(kt, 128, NB, 512).transpose(2, 1, 0, 3)
        )

    w1b = preblock(W1, W1_SCALE, f8)
    w2b = preblock(W2, W2_SCALE, f8) if fp8_b else preblock(W2, 1.0, bf)
    w3b = preblock(W3, W3_SCALE, f8)

    b1f = np.ascontiguousarray(b1).astype(np.float32) * np.float32(W1_SCALE)
    b2f = np.ascontiguousarray(b2).astype(np.float32)
    b3f = np.ascontiguousarray(b3).astype(np.float32)

    in_maps = []
    for i in range(NCORES):
        sl = slice(i * R, (i + 1) * R)
        xTc = xT[:, sl]       # [512, 512]
        xT8c = xT8[:, sl]
        adjc = adjT[:, sl].astype(f8)  # [4096, 512]
        in_maps.append(
            {
                # [128, kt, R] preblocked
                "xT": np.ascontiguousarray(
                    xTc.reshape(KT_IN, 128, R).transpose(1, 0, 2)),
                "xT8": np.ascontiguousarray(
                    xT8c.reshape(KT_IN, 128, R).transpose(1, 0, 2)),
                # [NB][128][4][R] preblocked
                "adjT": np.ascontiguousarray(
                    adjc.reshape(NB, 4, 128, R).transpose(0, 2, 1, 3)),
                "w1": w1b,
                "w2": w2b,
                "w3": w3b,
                "b1": b1f,
                "b2": b2f,
                "b3": b3f,
            }
        )
    return in_maps


def run(x, adj, W1, b1, W2, b2, W3, b3, trace=False, fp8_d=True, fp8_b=True):
    zb = not (np.any(np.asarray(b1)) or np.any(np.asarray(b3)))
    nc = build_nc(with_b2=bool(np.any(np.asarray(b2))), fp8_d=fp8_d, fp8_b=fp8_b,
                  zero_bias=zb)
    in_maps = make_in_maps(x, adj, W1, b1, W2, b2, W3, b3, fp8_b=fp8_b)
    res = run_bass_kernel_spmd(nc, in_maps, core_ids=list(range(NCORES)), trace=trace)
    out = np.concatenate([res.results[i]["out"] for i in range(NCORES)], axis=0)
    return out.astype(np.float32), res


def kernel(x, adj, W1, b1, W2, b2, W3, b3):
    args = [np.asarray(a) for a in (x, adj, W1, b1, W2, b2, W3, b3)]
    out, _ = run(*args, trace=False)
    return out
